# revision 22
# baseline (speedup 1.0000x reference)
"""Distributed MHA kernel for Trainium2 (8 NeuronCores).

Problem: x,f:(2,2048,1024), W_qkv:(1024,3072), W_proj:(1024,1024), H=16 heads.
reference returns (out, attn2gcn) with
  attn2gcn = softmax(q k^T / sqrt(64)) v   (per head, concat over heads)
  out      = (attn2gcn + f) @ W_proj + b_proj

Sharding: tensor-parallel over heads — core c owns heads 2c, 2c+1 for both
batches (column block c*128 of the hidden dim).  Matmul operands are bf16
(fp32 PSUM accumulation; x/W/f pre-cast to bf16 on the host so loads ride
the fast HWDGE queues); softmax arithmetic stays fp32.

Per-core dataflow, engineered to keep the PE busy in long contiguous
bursts (the HAM activity throttler halves the PE clock whenever it sees
idle windows, so every dependency stall is also a 2x clock penalty):
  qkvT = W-slice^T @ x^T as 3 M-tiles [k01 | q01 | v01] -> qT/kT
  head-packed tiles [128 = 2 heads x 64d, 4096 = (b,n)]; v transposed
  per n-tile (PE) into av-ready [kj, 64d | ones] blocks.  Only the first
  half of qkv (batch-0 columns) runs up front; the batch-1 half is
  emitted inside head 0's first attention chunks as PE filler.
  Attention per (head, batch) in scoresT layout: scoresT[kj, qi] = k q^T,
  software-pipelined (scores for kj+1 issue before the av matmuls of kj)
  so the PE runs ahead of the ACT exp; av^T accumulates with an all-ones
  65th v column giving the softmax denominator for free.  Normalization
  (fast-approx reciprocal + ones-broadcast matmul + DVE mult) for chunk
  c-1 is emitted inside chunk c, off the PE critical path; fusedT =
  avn + f^T (bf16), staged per chunk for the AllToAll.
  Row-resharding via one 8-rank AllToAll split in two bf16 halves; the
  head-0 half fires mid-kernel and its half-contraction of the output
  projection (K=64 tiles, bias folded in) interleaves into head 1's
  chunks as PE filler; the head-1 half completes the contraction at the
  end.
Host only transposes/concatenates per-core outputs (pure data movement).
"""

import numpy as np

B, N, C, H, D = 2, 2048, 1024, 16, 64
BN = B * N
SCALE = D ** -0.5
N_CORES = 8
KT = C // 128      # 8 contraction tiles
NCH = BN // 512    # 8 qkv free chunks

_cached = None


def _build():
    from contextlib import ExitStack

    import concourse.mybir as mybir
    import concourse.tile as tile
    from concourse import bacc
    from concourse.masks import make_identity

    F32 = mybir.dt.float32
    BF16 = mybir.dt.bfloat16
    EXP = mybir.ActivationFunctionType.Exp

    nc = bacc.Bacc("TRN2", target_bir_lowering=False, debug=False,
                   num_devices=N_CORES)

    xT_ext = nc.dram_tensor("xT", [C, BN], BF16, kind="ExternalInput").ap()
    wkqv_ext = nc.dram_tensor("wkqv", [C, 384], BF16, kind="ExternalInput").ap()
    fT_ext = nc.dram_tensor("fT", [128, BN], BF16, kind="ExternalInput").ap()
    wproj_ext = nc.dram_tensor("wproj", [C, C], BF16, kind="ExternalInput").ap()
    bprojT_ext = nc.dram_tensor("bprojT", [128, 8], F32, kind="ExternalInput").ap()
    attn_t_ext = nc.dram_tensor("attn_t", [128, BN], F32, kind="ExternalOutput").ap()
    out_t_ext = nc.dram_tensor("out_t", [C, 512], F32, kind="ExternalOutput").ap()

    groups = [list(range(N_CORES))]

    with tile.TileContext(nc) as tc:
        with ExitStack() as octx:
            pp = octx.enter_context(tc.tile_pool(name="persist", bufs=1))
            kqp = octx.enter_context(tc.tile_pool(name="kq", bufs=1))
            vap = octx.enter_context(tc.tile_pool(name="vaug", bufs=1))
            vtp = octx.enter_context(tc.tile_pool(name="vt", bufs=1))
            wqp = octx.enter_context(tc.tile_pool(name="wq", bufs=1))
            xsp = octx.enter_context(tc.tile_pool(name="xs", bufs=16))
            dram = octx.enter_context(
                tc.tile_pool(name="dram", bufs=1, space="DRAM"))
            # qkv weights first in the DMA queues — they gate the first matmul
            wq_sb = []
            for k in range(KT):
                w = wqp.tile([128, 384], BF16, name=f"wq{k}")
                nc.sync.dma_start(w[:], wkqv_ext[k * 128:(k + 1) * 128, :])
                wq_sb.append(w)

            ident = pp.tile([128, 128], BF16)
            make_identity(nc, ident[:])
            ones64b = pp.tile([1, 64], BF16)
            nc.vector.memset(ones64b[:], 1.0)

            kT = kqp.tile([128, BN], BF16, name="kT")
            qT = kqp.tile([128, BN], BF16, name="qT")
            vT = vtp.tile([128, BN], BF16, name="vT")
            mtiles = [kT, qT, vT]
            fusedT = [pp.tile([64, BN], BF16, name=f"fusedT{hh}")
                      for hh in range(2)]
            v_aug = [[vap.tile([128, 65], BF16, name=f"va{h}_{j}")
                      for j in range(32)] for h in range(2)]

            a2a_in = [dram.tile([512, 512], BF16, name=f"a2ain{hh}")
                      for hh in range(2)]
            a2a_out = [dram.tile([512, 512], BF16, name=f"a2aout{hh}")
                       for hh in range(2)]

            def qkv_transposes(nch, tpool, ttag):
                for j in range(4 * nch, 4 * nch + 4):
                    tps = tpool.tile([128, 128], BF16, name="tps", tag=ttag)
                    nc.tensor.transpose(
                        tps[:], vT[:, j * 128:(j + 1) * 128], ident[:])
                    for h in range(2):
                        nc.vector.tensor_copy(
                            v_aug[h][j][:, 0:64], tps[:, h * 64:(h + 1) * 64])
                        nc.vector.memset(v_aug[h][j][:, 64:65], 1.0)

            def qkv_xs(nch):
                xs_t = []
                for k in range(KT):
                    xs = xsp.tile([128, 512], BF16, name="xs", tag="xs")
                    nc.sync.dma_start(
                        xs[:], xT_ext[k * 128:(k + 1) * 128,
                                      nch * 512:(nch + 1) * 512])
                    xs_t.append(xs)
                return xs_t

            # ------------- phase Q: qkv for the first two chunks -------------
            with ExitStack() as qctx:
                qps = qctx.enter_context(
                    tc.tile_pool(name="qkv_ps", bufs=1, space="PSUM"))
                trp = qctx.enter_context(
                    tc.tile_pool(name="tr_ps", bufs=2, space="PSUM"))
                for nch in range(2):
                    xs_t = qkv_xs(nch)
                    psums = [qps.tile([128, 512], F32, name=f"qps{m}",
                                      tag=f"qps{m}") for m in range(3)]
                    for k in range(KT):
                        for m in range(3):
                            nc.tensor.matmul(
                                psums[m][:],
                                wq_sb[k][:, m * 128:(m + 1) * 128],
                                xs_t[k][:], start=(k == 0), stop=(k == KT - 1))
                    for m in range(3):
                        nc.vector.tensor_copy(
                            mtiles[m][:, nch * 512:(nch + 1) * 512],
                            psums[m][:])
                    qkv_transposes(nch, trp, "tps")

            # remaining f/bias loads (needed from normalization time on)
            bias_sb = pp.tile([128, 8], F32)
            nc.sync.dma_start(bias_sb[:], bprojT_ext[:])
            fT_sb = [pp.tile([64, BN], BF16, name=f"fTsb{hh}")
                     for hh in range(2)]
            for hh in range(2):
                nc.sync.dma_start(fT_sb[hh][:], fT_ext[hh * 64:(hh + 1) * 64, :])

            # ---------------- phase A: attention + fillers ----------------
            with ExitStack() as actx:
                expp = actx.enter_context(tc.tile_pool(name="exp", bufs=4))
                avup = actx.enter_context(tc.tile_pool(name="avu", bufs=3))
                normp = actx.enter_context(tc.tile_pool(name="norm", bufs=2))
                avnp = actx.enter_context(tc.tile_pool(name="avn", bufs=2))
                oaccp = actx.enter_context(tc.tile_pool(name="oacc", bufs=1))
                wp1p = actx.enter_context(tc.tile_pool(name="wp1", bufs=1))
                wp2p = actx.enter_context(tc.tile_pool(name="wp2", bufs=1))
                rhs1p = actx.enter_context(tc.tile_pool(name="rhs1", bufs=1))
                sps = actx.enter_context(
                    tc.tile_pool(name="scores_ps", bufs=2, space="PSUM"))
                avps = actx.enter_context(
                    tc.tile_pool(name="av_ps", bufs=1, space="PSUM"))
                bcps = actx.enter_context(
                    tc.tile_pool(name="bc_ps", bufs=1, space="PSUM"))
                pjps = actx.enter_context(
                    tc.tile_pool(name="pj_ps", bufs=1, space="PSUM"))
                out_acc = [oaccp.tile([128, 512], F32, name=f"oacc{m}")
                           for m in range(8)]
                # prefetch all projection weights during attention
                wp1_sb, wp2_sb = [], []
                for t in range(KT):
                    w1 = wp1p.tile([64, C], BF16, name=f"wp1_{t}")
                    nc.sync.dma_start(w1[:], wproj_ext[t * 128:t * 128 + 64, :])
                    wp1_sb.append(w1)
                    w2 = wp2p.tile([64, C], BF16, name=f"wp2_{t}")
                    nc.sync.dma_start(
                        w2[:], wproj_ext[t * 128 + 64:(t + 1) * 128, :])
                    wp2_sb.append(w2)

                rhs1_sb = []
                proj1_work = []
                qkv_work = [(nch, m) for nch in range(2, 8) for m in range(3)]

                qkv_xs_cache = {}

                def qkv_deferred_unit(unit):
                    """One m-pass of a deferred qkv chunk — small PE
                    filler emitted inside head 0's early attention chunks
                    (single pj psum slot)."""
                    nch, m = unit
                    if nch not in qkv_xs_cache:
                        qkv_xs_cache[nch] = qkv_xs(nch)
                    xs_t = qkv_xs_cache[nch]
                    pjt = pjps.tile([128, 512], F32, name="qkvd", tag="pj")
                    for k in range(KT):
                        nc.tensor.matmul(
                            pjt[:], wq_sb[k][:, m * 128:(m + 1) * 128],
                            xs_t[k][:], start=(k == 0), stop=(k == KT - 1))
                    nc.vector.tensor_copy(
                        mtiles[m][:, nch * 512:(nch + 1) * 512], pjt[:])
                    if m == 2:
                        qkv_transposes(nch, bcps, "bc")
                        del qkv_xs_cache[nch]

                def proj1_m(m):
                    pj = pjps.tile([128, 512], F32, name="pj", tag="pj")
                    for t in range(KT):
                        nc.tensor.matmul(
                            pj[:], wp1_sb[t][:, m * 128:(m + 1) * 128],
                            rhs1_sb[t][:], start=(t == 0), stop=(t == KT - 1))
                    nc.vector.tensor_scalar_add(
                        out_acc[m][:], pj[:], bias_sb[:, m:m + 1])

                def norm_pre(avu):
                    """1/denom chain — latency starts at chunk end."""
                    dn = normp.tile([1, 1024], F32, name="dn", tag="dn")
                    nc.sync.dma_start(dn[:], avu[64:65, :])
                    dninv = normp.tile([1, 1024], F32, name="dninv",
                                       tag="dninv")
                    nc.vector.reciprocal_approx_fast(dninv[:], dn[:])
                    dinvb = normp.tile([1, 1024], BF16, name="dinvb",
                                       tag="dinvb")
                    nc.vector.tensor_copy(dinvb[:], dninv[:])
                    return dinvb

                def norm_chunk(h, b, ch, avu, dinvb):
                    """avn = avu[0:64]/avu[64]; attn_t out; fusedT; staging."""
                    po = h * 64
                    cs = b * 2048 + ch * 1024
                    avn = avnp.tile([64, 1024], F32, name="avn", tag="avn")
                    for s in range(2):
                        bc = bcps.tile([64, 512], F32, name="bc", tag="bc")
                        nc.tensor.matmul(bc[:], ones64b[:],
                                         dinvb[:, s * 512:(s + 1) * 512],
                                         start=True, stop=True)
                        nc.vector.tensor_tensor(
                            avn[:, s * 512:(s + 1) * 512],
                            avu[0:64, s * 512:(s + 1) * 512],
                            bc[:], mybir.AluOpType.mult)
                    nc.sync.dma_start(
                        attn_t_ext[po:po + 64, cs:cs + 1024], avn[:])
                    nc.vector.tensor_tensor(
                        fusedT[h][:, cs:cs + 1024], avn[:],
                        fT_sb[h][:, cs:cs + 1024], mybir.AluOpType.add)
                    for j in (cs // 512, cs // 512 + 1):
                        nc.sync.dma_start(
                            a2a_in[h][j * 64:(j + 1) * 64, :],
                            fusedT[h][:, j * 512:(j + 1) * 512])

                def mm1_kj(h, b, cs, kj, scores_q):
                    po = h * 64
                    jt = b * 16 + kj
                    sc = sps.tile([128, 1024], F32, name="scores", tag="sc")
                    for s in range(2):
                        nc.tensor.matmul(
                            sc[:, s * 512:(s + 1) * 512],
                            kT[po:po + 64, jt * 128:(jt + 1) * 128],
                            qT[po:po + 64, cs + s * 512:cs + (s + 1) * 512],
                            start=True, stop=True)
                    scores_q[kj] = sc

                def fire_a2a(h):
                    nc.gpsimd.collective_compute(
                        "AllToAll", mybir.AluOpType.bypass,
                        replica_groups=groups,
                        ins=[a2a_in[h].opt()], outs=[a2a_out[h].opt()])
                    if h == 0:
                        for t in range(KT):
                            r = rhs1p.tile([64, 512], BF16, name=f"rhs1_{t}")
                            nc.sync.dma_start(
                                r[:], a2a_out[0][t * 64:(t + 1) * 64, :])
                            rhs1_sb.append(r)
                        proj1_work.extend(range(8))

                def do_chunk(h, b, ch, pending):
                    cs = b * 2048 + ch * 1024
                    av = avps.tile([128, 1024], F32, name="av", tag="av")
                    scores_q = {}
                    mm1_kj(h, b, cs, 0, scores_q)
                    for kj in range(16):
                        if kj + 1 < 16:
                            mm1_kj(h, b, cs, kj + 1, scores_q)
                        if kj == 2 and pending is not None:
                            ph = pending[0]
                            norm_chunk(*pending)
                            pending = None
                            if ph != h:
                                # that was the previous head's last chunk:
                                # its AllToAll half can fire now
                                fire_a2a(ph)
                        if h == 0 and b == 0 and kj % 2 == 1 and qkv_work:
                            qkv_deferred_unit(qkv_work.pop(0))
                            if kj <= 5 and qkv_work:
                                qkv_deferred_unit(qkv_work.pop(0))
                        if (kj == 9 and proj1_work
                                and not (b == 0 and ch == 0)):
                            proj1_m(proj1_work.pop(0))
                        sc = scores_q.pop(kj)
                        ex = expp.tile([128, 1024], BF16, name="ex", tag="ex")
                        nc.scalar.activation(ex[:], sc[:], EXP, scale=SCALE)
                        jt = b * 16 + kj
                        for s in range(2):
                            nc.tensor.matmul(
                                av[0:65, s * 512:(s + 1) * 512],
                                v_aug[h][jt][:],
                                ex[:, s * 512:(s + 1) * 512],
                                start=(kj == 0), stop=(kj == 15))
                    avu = avup.tile([65, 1024], F32, name="avu", tag="avu")
                    nc.vector.tensor_copy(avu[:], av[0:65, :])
                    return (h, b, ch, avu, norm_pre(avu))

                pending = None
                for h in range(2):
                    for b in range(2):
                        for ch in range(2):
                            pending = do_chunk(h, b, ch, pending)
                # final chunk of head 1, then its collective
                norm_chunk(*pending)
                fire_a2a(1)

                # remaining proj1 m-tiles: PE filler under AllToAll #2
                while proj1_work:
                    proj1_m(proj1_work.pop(0))

                # ---------------- proj phase 2 ----------------
                with ExitStack() as pctx:
                    rhs2p = pctx.enter_context(
                        tc.tile_pool(name="rhs2", bufs=1))
                    outp = pctx.enter_context(
                        tc.tile_pool(name="outp", bufs=2))
                    rhs2_sb = []
                    for t in range(KT):
                        r = rhs2p.tile([64, 512], BF16, name=f"rhs2_{t}")
                        nc.sync.dma_start(
                            r[:], a2a_out[1][t * 64:(t + 1) * 64, :])
                        rhs2_sb.append(r)
                    for m in range(8):
                        pj = pjps.tile([128, 512], F32, name="pj2", tag="pj")
                        for t in range(KT):
                            nc.tensor.matmul(
                                pj[:], wp2_sb[t][:, m * 128:(m + 1) * 128],
                                rhs2_sb[t][:], start=(t == 0),
                                stop=(t == KT - 1))
                        ot = outp.tile([128, 512], F32, name="ot", tag="ot")
                        nc.vector.tensor_tensor(ot[:], pj[:], out_acc[m][:],
                                                mybir.AluOpType.add)
                        nc.sync.dma_start(
                            out_t_ext[m * 128:(m + 1) * 128, :], ot[:])

    nc.compile()
    return nc


def kernel(x, f, W_qkv, W_proj, b_proj):
    import ml_dtypes
    from concourse.bass_utils import run_bass_kernel_spmd

    global _cached
    if _cached is None:
        _cached = _build()
    nc = _cached

    BF = ml_dtypes.bfloat16
    x = np.ascontiguousarray(np.asarray(x, dtype=np.float32))
    f = np.ascontiguousarray(np.asarray(f, dtype=np.float32))
    W_qkv = np.asarray(W_qkv, dtype=np.float32)
    W_proj = np.asarray(W_proj, dtype=np.float32)
    b_proj = np.asarray(b_proj, dtype=np.float32)

    Wq, Wk, Wv = W_qkv[:, 0:C], W_qkv[:, C:2 * C], W_qkv[:, 2 * C:3 * C]
    xT = np.ascontiguousarray(x.reshape(BN, C).T.astype(BF))
    fT = np.ascontiguousarray(f.reshape(BN, C).T.astype(BF))
    wproj_b = np.ascontiguousarray(W_proj.astype(BF))
    bprojT = np.ascontiguousarray(b_proj.reshape(8, 128).T)

    in_maps = []
    for c in range(N_CORES):
        cols = slice(c * 128, (c + 1) * 128)     # heads 2c, 2c+1
        wkqv = np.ascontiguousarray(np.concatenate(
            [Wk[:, cols], Wq[:, cols], Wv[:, cols]], axis=1).astype(BF))
        in_maps.append({
            "xT": xT,
            "wkqv": wkqv,
            "fT": np.ascontiguousarray(fT[cols, :]),
            "wproj": wproj_b,
            "bprojT": bprojT,
        })

    res = run_bass_kernel_spmd(nc, in_maps, core_ids=list(range(N_CORES)))

    attn = np.empty((BN, C), dtype=np.float32)
    out = np.empty((BN, C), dtype=np.float32)
    for c in range(N_CORES):
        r = res.results[c]
        attn[:, c * 128:(c + 1) * 128] = r["attn_t"].T
        out[c * 512:(c + 1) * 512, :] = r["out_t"].T
    return out.reshape(B, N, C), attn.reshape(B, N, C)


# revision 23
# speedup vs baseline: 1.0384x; 1.0384x over previous
"""Distributed MHA kernel for Trainium2 (8 NeuronCores).

Problem: x,f:(2,2048,1024), W_qkv:(1024,3072), W_proj:(1024,1024), H=16 heads.
reference returns (out, attn2gcn) with
  attn2gcn = softmax(q k^T / sqrt(64)) v   (per head, concat over heads)
  out      = (attn2gcn + f) @ W_proj + b_proj

Sharding: tensor-parallel over heads — core c owns heads 2c, 2c+1 for both
batches (column block c*128 of the hidden dim).  Matmul operands are bf16
(fp32 PSUM accumulation; x/W/f pre-cast to bf16 on the host so loads ride
the fast HWDGE queues); softmax arithmetic stays fp32.

Per-core dataflow, engineered to keep the PE busy in long contiguous
bursts (the HAM activity throttler halves the PE clock whenever it sees
idle windows, so every dependency stall is also a 2x clock penalty):
  qkvT = W-slice^T @ x^T as 3 M-tiles [k01 | q01 | v01] -> qT/kT
  head-packed tiles [128 = 2 heads x 64d, 4096 = (b,n)]; v transposed
  per n-tile (PE) into av-ready [kj, 64d | ones] blocks.  Only the first
  half of qkv (batch-0 columns) runs up front; the batch-1 half is
  emitted inside head 0's first attention chunks as PE filler.
  Attention per (head, batch) in scoresT layout: scoresT[kj, qi] = k q^T,
  software-pipelined (scores for kj+1 issue before the av matmuls of kj)
  so the PE runs ahead of the ACT exp; av^T accumulates with an all-ones
  65th v column giving the softmax denominator for free.  Normalization
  (fast-approx reciprocal + ones-broadcast matmul + DVE mult) for chunk
  c-1 is emitted inside chunk c, off the PE critical path; fusedT =
  avn + f^T (bf16), staged per chunk for the AllToAll.
  Row-resharding via one 8-rank AllToAll split in two bf16 halves; the
  head-0 half fires mid-kernel and its half-contraction of the output
  projection (K=64 tiles, bias folded in) interleaves into head 1's
  chunks as PE filler; the head-1 half completes the contraction at the
  end.
Host only transposes/concatenates per-core outputs (pure data movement).
"""

import numpy as np

B, N, C, H, D = 2, 2048, 1024, 16, 64
BN = B * N
SCALE = D ** -0.5
N_CORES = 8
KT = C // 128      # 8 contraction tiles
NCH = BN // 512    # 8 qkv free chunks

_cached = None


def _build():
    from contextlib import ExitStack

    import concourse.mybir as mybir
    import concourse.tile as tile
    from concourse import bacc
    from concourse.masks import make_identity

    F32 = mybir.dt.float32
    BF16 = mybir.dt.bfloat16
    EXP = mybir.ActivationFunctionType.Exp

    nc = bacc.Bacc("TRN2", target_bir_lowering=False, debug=False,
                   num_devices=N_CORES)

    xT_ext = nc.dram_tensor("xT", [C, BN], BF16, kind="ExternalInput").ap()
    wkqv_ext = nc.dram_tensor("wkqv", [C, 384], BF16, kind="ExternalInput").ap()
    fT_ext = nc.dram_tensor("fT", [128, BN], BF16, kind="ExternalInput").ap()
    wproj_ext = nc.dram_tensor("wproj", [C, C], BF16, kind="ExternalInput").ap()
    bprojT_ext = nc.dram_tensor("bprojT", [128, 8], F32, kind="ExternalInput").ap()
    attn_t_ext = nc.dram_tensor("attn_t", [128, BN], F32, kind="ExternalOutput").ap()
    out_t_ext = nc.dram_tensor("out_t", [C, 512], F32, kind="ExternalOutput").ap()

    groups = [list(range(N_CORES))]

    with tile.TileContext(nc) as tc:
        with ExitStack() as octx:
            pp = octx.enter_context(tc.tile_pool(name="persist", bufs=1))
            kqp = octx.enter_context(tc.tile_pool(name="kq", bufs=1))
            vap = octx.enter_context(tc.tile_pool(name="vaug", bufs=1))
            vtp = octx.enter_context(tc.tile_pool(name="vt", bufs=1))
            wqp = octx.enter_context(tc.tile_pool(name="wq", bufs=1))
            xsp = octx.enter_context(tc.tile_pool(name="xs", bufs=16))
            dram = octx.enter_context(
                tc.tile_pool(name="dram", bufs=1, space="DRAM"))
            # qkv weights first in the DMA queues — they gate the first matmul
            wq_sb = []
            for k in range(KT):
                w = wqp.tile([128, 384], BF16, name=f"wq{k}")
                nc.sync.dma_start(w[:], wkqv_ext[k * 128:(k + 1) * 128, :])
                wq_sb.append(w)

            ident = pp.tile([128, 128], BF16)
            make_identity(nc, ident[:])
            ones64b = pp.tile([1, 64], BF16)
            nc.vector.memset(ones64b[:], 1.0)

            kT = kqp.tile([128, BN], BF16, name="kT")
            qT = kqp.tile([128, BN], BF16, name="qT")
            vT = vtp.tile([128, BN], BF16, name="vT")
            mtiles = [kT, qT, vT]
            fusedT = [pp.tile([64, BN], BF16, name=f"fusedT{hh}")
                      for hh in range(2)]
            v_aug = [[vap.tile([128, 65], BF16, name=f"va{h}_{j}")
                      for j in range(32)] for h in range(2)]

            a2a_in = [dram.tile([512, 512], BF16, name=f"a2ain{hh}")
                      for hh in range(2)]
            a2a_out = [dram.tile([512, 512], BF16, name=f"a2aout{hh}")
                       for hh in range(2)]

            def qkv_transposes(nch, tpool, ttag):
                for j in range(4 * nch, 4 * nch + 4):
                    tps = tpool.tile([128, 128], BF16, name="tps", tag=ttag)
                    nc.tensor.transpose(
                        tps[:], vT[:, j * 128:(j + 1) * 128], ident[:])
                    for h in range(2):
                        nc.vector.tensor_copy(
                            v_aug[h][j][:, 0:64], tps[:, h * 64:(h + 1) * 64])
                        nc.vector.memset(v_aug[h][j][:, 64:65], 1.0)

            def qkv_xs(nch):
                xs_t = []
                for k in range(KT):
                    xs = xsp.tile([128, 512], BF16, name="xs", tag="xs")
                    nc.sync.dma_start(
                        xs[:], xT_ext[k * 128:(k + 1) * 128,
                                      nch * 512:(nch + 1) * 512])
                    xs_t.append(xs)
                return xs_t

            # ------------- phase Q: qkv for the first two chunks -------------
            with ExitStack() as qctx:
                qps = qctx.enter_context(
                    tc.tile_pool(name="qkv_ps", bufs=1, space="PSUM"))
                trp = qctx.enter_context(
                    tc.tile_pool(name="tr_ps", bufs=2, space="PSUM"))
                for nch in range(2):
                    xs_t = qkv_xs(nch)
                    psums = [qps.tile([128, 512], F32, name=f"qps{m}",
                                      tag=f"qps{m}") for m in range(3)]
                    for k in range(KT):
                        for m in range(3):
                            nc.tensor.matmul(
                                psums[m][:],
                                wq_sb[k][:, m * 128:(m + 1) * 128],
                                xs_t[k][:], start=(k == 0), stop=(k == KT - 1))
                    for m in range(3):
                        nc.vector.tensor_copy(
                            mtiles[m][:, nch * 512:(nch + 1) * 512],
                            psums[m][:])
                    qkv_transposes(nch, trp, "tps")

            # remaining f/bias loads (needed from normalization time on)
            bias_sb = pp.tile([128, 8], F32)
            nc.sync.dma_start(bias_sb[:], bprojT_ext[:])
            fT_sb = [pp.tile([64, BN], BF16, name=f"fTsb{hh}")
                     for hh in range(2)]
            for hh in range(2):
                nc.sync.dma_start(fT_sb[hh][:], fT_ext[hh * 64:(hh + 1) * 64, :])

            # ---------------- phase A: attention + fillers ----------------
            with ExitStack() as actx:
                expp = actx.enter_context(tc.tile_pool(name="exp", bufs=4))
                avup = actx.enter_context(tc.tile_pool(name="avu", bufs=3))
                normp = actx.enter_context(tc.tile_pool(name="norm", bufs=2))
                avnp = actx.enter_context(tc.tile_pool(name="avn", bufs=2))
                oaccp = actx.enter_context(tc.tile_pool(name="oacc", bufs=1))
                wp1p = actx.enter_context(tc.tile_pool(name="wp1", bufs=1))
                wp2p = actx.enter_context(tc.tile_pool(name="wp2", bufs=1))
                rhs1p = actx.enter_context(tc.tile_pool(name="rhs1", bufs=1))
                sps = actx.enter_context(
                    tc.tile_pool(name="scores_ps", bufs=2, space="PSUM"))
                avps = actx.enter_context(
                    tc.tile_pool(name="av_ps", bufs=1, space="PSUM"))
                bcps = actx.enter_context(
                    tc.tile_pool(name="bc_ps", bufs=1, space="PSUM"))
                pjps = actx.enter_context(
                    tc.tile_pool(name="pj_ps", bufs=1, space="PSUM"))
                out_acc = [oaccp.tile([128, 512], F32, name=f"oacc{m}")
                           for m in range(8)]
                # prefetch all projection weights during attention
                wp1_sb, wp2_sb = [], []
                for t in range(KT):
                    w1 = wp1p.tile([64, C], BF16, name=f"wp1_{t}")
                    nc.sync.dma_start(w1[:], wproj_ext[t * 128:t * 128 + 64, :])
                    wp1_sb.append(w1)
                    w2 = wp2p.tile([64, C], BF16, name=f"wp2_{t}")
                    nc.sync.dma_start(
                        w2[:], wproj_ext[t * 128 + 64:(t + 1) * 128, :])
                    wp2_sb.append(w2)

                rhs1_sb = []
                proj1_work = []
                qkv_work = [(nch, m) for nch in range(2, 8) for m in range(3)]

                qkv_xs_cache = {}

                def qkv_deferred_unit(unit):
                    """One m-pass of a deferred qkv chunk — small PE
                    filler emitted inside head 0's early attention chunks
                    (single pj psum slot)."""
                    nch, m = unit
                    if nch not in qkv_xs_cache:
                        qkv_xs_cache[nch] = qkv_xs(nch)
                    xs_t = qkv_xs_cache[nch]
                    pjt = pjps.tile([128, 512], F32, name="qkvd", tag="pj")
                    for k in range(KT):
                        nc.tensor.matmul(
                            pjt[:], wq_sb[k][:, m * 128:(m + 1) * 128],
                            xs_t[k][:], start=(k == 0), stop=(k == KT - 1))
                    nc.vector.tensor_copy(
                        mtiles[m][:, nch * 512:(nch + 1) * 512], pjt[:])
                    if m == 2:
                        qkv_transposes(nch, bcps, "bc")
                        del qkv_xs_cache[nch]

                def proj1_m(m):
                    pj = pjps.tile([128, 512], F32, name="pj", tag="pj")
                    for t in range(KT):
                        nc.tensor.matmul(
                            pj[:], wp1_sb[t][:, m * 128:(m + 1) * 128],
                            rhs1_sb[t][:], start=(t == 0), stop=(t == KT - 1))
                    nc.vector.tensor_scalar_add(
                        out_acc[m][:], pj[:], bias_sb[:, m:m + 1])

                def norm_pre(avu):
                    """1/denom chain — latency starts at chunk end."""
                    dn = normp.tile([1, 1024], F32, name="dn", tag="dn")
                    nc.sync.dma_start(dn[:], avu[64:65, :])
                    dninv = normp.tile([1, 1024], F32, name="dninv",
                                       tag="dninv")
                    nc.vector.reciprocal_approx_fast(dninv[:], dn[:])
                    dinvb = normp.tile([1, 1024], BF16, name="dinvb",
                                       tag="dinvb")
                    nc.vector.tensor_copy(dinvb[:], dninv[:])
                    return dinvb

                def norm_chunk(h, b, ch, avu, dinvb):
                    """avn = avu[0:64]/avu[64]; attn_t out; fusedT; staging."""
                    po = h * 64
                    cs = b * 2048 + ch * 1024
                    avn = avnp.tile([64, 1024], F32, name="avn", tag="avn")
                    for s in range(2):
                        bc = bcps.tile([64, 512], F32, name="bc", tag="bc")
                        nc.tensor.matmul(bc[:], ones64b[:],
                                         dinvb[:, s * 512:(s + 1) * 512],
                                         start=True, stop=True)
                        nc.vector.tensor_tensor(
                            avn[:, s * 512:(s + 1) * 512],
                            avu[0:64, s * 512:(s + 1) * 512],
                            bc[:], mybir.AluOpType.mult)
                    nc.sync.dma_start(
                        attn_t_ext[po:po + 64, cs:cs + 1024], avn[:])
                    nc.vector.tensor_tensor(
                        fusedT[h][:, cs:cs + 1024], avn[:],
                        fT_sb[h][:, cs:cs + 1024], mybir.AluOpType.add)
                    for j in (cs // 512, cs // 512 + 1):
                        nc.sync.dma_start(
                            a2a_in[h][j * 64:(j + 1) * 64, :],
                            fusedT[h][:, j * 512:(j + 1) * 512])

                def mm1_kj(h, b, cs, kj, scores_q):
                    po = h * 64
                    jt = b * 16 + kj
                    sc = sps.tile([128, 1024], F32, name="scores", tag="sc")
                    for s in range(2):
                        nc.tensor.matmul(
                            sc[:, s * 512:(s + 1) * 512],
                            kT[po:po + 64, jt * 128:(jt + 1) * 128],
                            qT[po:po + 64, cs + s * 512:cs + (s + 1) * 512],
                            start=True, stop=True)
                    scores_q[kj] = sc

                def fire_a2a(h):
                    nc.gpsimd.collective_compute(
                        "AllToAll", mybir.AluOpType.bypass,
                        replica_groups=groups,
                        ins=[a2a_in[h].opt()], outs=[a2a_out[h].opt()])
                    if h == 0:
                        for t in range(KT):
                            r = rhs1p.tile([64, 512], BF16, name=f"rhs1_{t}")
                            nc.sync.dma_start(
                                r[:], a2a_out[0][t * 64:(t + 1) * 64, :])
                            rhs1_sb.append(r)
                        proj1_work.extend(range(8))

                def do_chunk(h, b, ch, pending):
                    cs = b * 2048 + ch * 1024
                    av = avps.tile([128, 1024], F32, name="av", tag="av")
                    scores_q = {}
                    mm1_kj(h, b, cs, 0, scores_q)
                    for kj in range(16):
                        if kj + 1 < 16:
                            mm1_kj(h, b, cs, kj + 1, scores_q)
                        if kj == 2 and pending is not None:
                            ph = pending[0]
                            norm_chunk(*pending)
                            pending = None
                            if ph != h:
                                # that was the previous head's last chunk:
                                # its AllToAll half can fire now
                                fire_a2a(ph)
                        if (h == 0 and not (b == 1 and ch == 1)
                                and kj in (1, 3, 5) and qkv_work):
                            qkv_deferred_unit(qkv_work.pop(0))
                            if qkv_work:
                                qkv_deferred_unit(qkv_work.pop(0))
                        if (kj == 9 and proj1_work
                                and not (b == 0 and ch == 0)):
                            proj1_m(proj1_work.pop(0))
                        sc = scores_q.pop(kj)
                        ex = expp.tile([128, 1024], BF16, name="ex", tag="ex")
                        nc.scalar.activation(ex[:], sc[:], EXP, scale=SCALE)
                        jt = b * 16 + kj
                        for s in range(2):
                            nc.tensor.matmul(
                                av[0:65, s * 512:(s + 1) * 512],
                                v_aug[h][jt][:],
                                ex[:, s * 512:(s + 1) * 512],
                                start=(kj == 0), stop=(kj == 15))
                    avu = avup.tile([65, 1024], F32, name="avu", tag="avu")
                    nc.vector.tensor_copy(avu[:], av[0:65, :])
                    return (h, b, ch, avu, norm_pre(avu))

                pending = None
                for h in range(2):
                    for b in range(2):
                        for ch in range(2):
                            pending = do_chunk(h, b, ch, pending)
                # final chunk of head 1, then its collective
                norm_chunk(*pending)
                fire_a2a(1)

                # remaining proj1 m-tiles: PE filler under AllToAll #2
                while proj1_work:
                    proj1_m(proj1_work.pop(0))

                # ---------------- proj phase 2 ----------------
                with ExitStack() as pctx:
                    rhs2p = pctx.enter_context(
                        tc.tile_pool(name="rhs2", bufs=1))
                    outp = pctx.enter_context(
                        tc.tile_pool(name="outp", bufs=2))
                    rhs2_sb = []
                    for t in range(KT):
                        r = rhs2p.tile([64, 512], BF16, name=f"rhs2_{t}")
                        nc.sync.dma_start(
                            r[:], a2a_out[1][t * 64:(t + 1) * 64, :])
                        rhs2_sb.append(r)
                    for m in range(8):
                        pj = pjps.tile([128, 512], F32, name="pj2", tag="pj")
                        for t in range(KT):
                            nc.tensor.matmul(
                                pj[:], wp2_sb[t][:, m * 128:(m + 1) * 128],
                                rhs2_sb[t][:], start=(t == 0),
                                stop=(t == KT - 1))
                        ot = outp.tile([128, 512], F32, name="ot", tag="ot")
                        nc.vector.tensor_tensor(ot[:], pj[:], out_acc[m][:],
                                                mybir.AluOpType.add)
                        nc.sync.dma_start(
                            out_t_ext[m * 128:(m + 1) * 128, :], ot[:])

    nc.compile()
    return nc


def kernel(x, f, W_qkv, W_proj, b_proj):
    import ml_dtypes
    from concourse.bass_utils import run_bass_kernel_spmd

    global _cached
    if _cached is None:
        _cached = _build()
    nc = _cached

    BF = ml_dtypes.bfloat16
    x = np.ascontiguousarray(np.asarray(x, dtype=np.float32))
    f = np.ascontiguousarray(np.asarray(f, dtype=np.float32))
    W_qkv = np.asarray(W_qkv, dtype=np.float32)
    W_proj = np.asarray(W_proj, dtype=np.float32)
    b_proj = np.asarray(b_proj, dtype=np.float32)

    Wq, Wk, Wv = W_qkv[:, 0:C], W_qkv[:, C:2 * C], W_qkv[:, 2 * C:3 * C]
    xT = np.ascontiguousarray(x.reshape(BN, C).T.astype(BF))
    fT = np.ascontiguousarray(f.reshape(BN, C).T.astype(BF))
    wproj_b = np.ascontiguousarray(W_proj.astype(BF))
    bprojT = np.ascontiguousarray(b_proj.reshape(8, 128).T)

    in_maps = []
    for c in range(N_CORES):
        cols = slice(c * 128, (c + 1) * 128)     # heads 2c, 2c+1
        wkqv = np.ascontiguousarray(np.concatenate(
            [Wk[:, cols], Wq[:, cols], Wv[:, cols]], axis=1).astype(BF))
        in_maps.append({
            "xT": xT,
            "wkqv": wkqv,
            "fT": np.ascontiguousarray(fT[cols, :]),
            "wproj": wproj_b,
            "bprojT": bprojT,
        })

    res = run_bass_kernel_spmd(nc, in_maps, core_ids=list(range(N_CORES)))

    attn = np.empty((BN, C), dtype=np.float32)
    out = np.empty((BN, C), dtype=np.float32)
    for c in range(N_CORES):
        r = res.results[c]
        attn[:, c * 128:(c + 1) * 128] = r["attn_t"].T
        out[c * 512:(c + 1) * 512, :] = r["out_t"].T
    return out.reshape(B, N, C), attn.reshape(B, N, C)
